# revision 9
# baseline (speedup 1.0000x reference)
"""Sharded gated-attention kernel for 8 Trainium2 NeuronCores.

Reference computation (per batch b):
    Q/K/V = x @ W{q,k,v}.T + b{q,k,v}   (16 heads of size 64)
    scores = (Q K^T) * sm_scale * M[b]          (gate pre-softmax)
    attn   = softmax(scores)
    A      = attn * M[b]                         (gate post-softmax)
    A_bar  = A / (sum_k |A| + eps)               (L1 renorm; A >= 0 here)
    out    = (A_bar V) @ Wo.T + bo

Math simplification: with e = exp(scores * M), softmax's 1/sum(e)
cancels in the renormalization, so A_bar = (e*M) / sum_k(e*M).  The eps
term is relatively ~1e-8 and dropped.  exp needs no max-subtraction:
|scores*M| <= ~7 for this distribution, comfortably in fp16/fp32 range.

Sharding: core c handles batch b = c // 4 and heads 4*(c%4)..4*(c%4)+3.
Each core computes a partial output (its heads' contribution through
Wo); the host sums the 4 partials per batch and adds bo.  No cross-core
communication.

Numerics: all big matmuls run with fp16 operands (1 cycle/row on the
PE, ~2^-11 operand rounding) accumulating in fp32 PSUM.  The softmax
pipeline is fp16 with fp32 row sums (via a ones column in V).

Per-core layouts (partition dim first):
    xsb  [128, 8, 2048]   x[b].T        (d-in-chunk, d-chunk, s)
    qt/kt[128, 2, 2048]   Q^T/K^T       (head-in-pair*64+hd, pair, s)
    v16  [128, 16, 4, 65] V | ones      (s-in-block, s-block, head, hd|1)
    scores are computed transposed, sT[k, q], so the softmax-k reduction
    rides the A@V matmul via the ones column.
"""

import numpy as np

B, S, D, H, HD = 2, 2048, 1024, 16, 64
P = 128
NCORES = 8
GROUPS = NCORES // B            # cores per batch = 4
HPC = H // GROUPS               # heads per core = 4
WROWS = HPC * HD                # weight rows per core = 256
NJ = S // 512                   # q blocks of 512
NI = S // P                     # k blocks of 128
ND = D // P                     # d chunks of 128

_CACHE = {}


def _build_nc():
    import concourse.bacc as bacc
    import concourse.mybir as mybir
    import concourse.tile as tile

    f32 = mybir.dt.float32
    f16 = mybir.dt.float16
    Alu = mybir.AluOpType
    Act = mybir.ActivationFunctionType

    nc = bacc.Bacc("TRN2", target_bir_lowering=False, debug=False,
                   num_devices=NCORES)

    xT = nc.declare_dram_parameter("xT", [ND, P, S], f16, isOutput=False).ap()
    m16 = nc.declare_dram_parameter("m16", [S, S], f16, isOutput=False).ap()
    wq = nc.declare_dram_parameter("wq", [ND, P, WROWS], f16, isOutput=False).ap()
    wk = nc.declare_dram_parameter("wk", [ND, P, WROWS], f16, isOutput=False).ap()
    wv = nc.declare_dram_parameter("wv", [ND, P, WROWS], f16, isOutput=False).ap()
    bq = nc.declare_dram_parameter("bq", [2, P], f32, isOutput=False).ap()
    bk = nc.declare_dram_parameter("bk", [2, P], f32, isOutput=False).ap()
    bv = nc.declare_dram_parameter("bv", [1, WROWS], f32, isOutput=False).ap()
    wo = nc.declare_dram_parameter("wo", [2, P, D], f16, isOutput=False).ap()
    out = nc.declare_dram_parameter("out", [S, D], f32, isOutput=True).ap()

    with tile.TileContext(nc) as tc:
        with (
            tc.tile_pool(name="const", bufs=1) as const,
            tc.tile_pool(name="qk_sb", bufs=1) as qk_sb,
            tc.tile_pool(name="v_pool", bufs=1) as v_pool,
            tc.tile_pool(name="y_pool", bufs=1) as y_pool,
        ):
            ones = const.tile([1, P], f32)
            nc.vector.memset(ones[:], 1.0)
            bq_sb = const.tile([P, 2], f32, tag="bq")
            bk_sb = const.tile([P, 2], f32, tag="bk")
            bv_sb = const.tile([1, WROWS], f32, tag="bv")
            nc.sync.dma_start(bq_sb[:], bq.rearrange("c p -> p c"))
            nc.sync.dma_start(bk_sb[:], bk.rearrange("c p -> p c"))
            nc.sync.dma_start(bv_sb[:], bv[:])

            qt = qk_sb.tile([P, 2, S], f16, tag="qt")
            kt = qk_sb.tile([P, 2, S], f16, tag="kt")
            v16 = v_pool.tile([P, NI, HPC, HD + 1], f16)
            nc.vector.memset(v16[:, :, :, HD:HD + 1], 1.0)
            y_sb = y_pool.tile([P, 2, S], f16)

            # ---------------- Phase A: projections ----------------
            with (
                tc.tile_pool(name="x_sb", bufs=1) as x_pool,
                tc.tile_pool(name="w_sb", bufs=1) as w_pool,
                tc.tile_pool(name="psA", bufs=2, space="PSUM") as psA,
            ):
                xsb = x_pool.tile([P, ND, S], f16)
                for d in range(ND):
                    nc.sync.dma_start(xsb[:, d, :], xT[d])
                wq_sb = w_pool.tile([P, ND, WROWS], f16, tag="wq")
                wk_sb = w_pool.tile([P, ND, WROWS], f16, tag="wk")
                wv_sb = w_pool.tile([P, ND, WROWS], f16, tag="wv")
                nc.sync.dma_start(wq_sb[:], wq.rearrange("c p w -> p c w"))
                nc.sync.dma_start(wk_sb[:], wk.rearrange("c p w -> p c w"))
                nc.sync.dma_start(wv_sb[:], wv.rearrange("c p w -> p c w"))

                # Q^T and K^T: [wrow-in-pair partition, s free]; bias fused
                # into the ACT PSUM-evacuation (per-partition bias).
                for w_sb, b_sb, dst in ((wq_sb, bq_sb, qt), (wk_sb, bk_sb, kt)):
                    for pp in range(2):
                        for j in range(NJ):
                            pt = psA.tile([P, 512], f32, tag="proj")
                            for d in range(ND):
                                nc.tensor.matmul(
                                    pt[:], w_sb[:, d, pp * P:(pp + 1) * P],
                                    xsb[:, d, j * 512:(j + 1) * 512],
                                    start=(d == 0), stop=(d == ND - 1))
                            nc.scalar.activation(dst[:, pp, j * 512:(j + 1) * 512],
                                                 pt[:], Act.Identity,
                                                 bias=b_sb[:, pp:pp + 1])

                # V natural layout [s partition, head dim free]; bias via a
                # K=1 ones matmul seeding PSUM; fp16 evac + ones column.
                for i in range(NI):
                    pv = psA.tile([P, WROWS], f32, tag="vproj")
                    nc.tensor.matmul(pv[:], ones[:], bv_sb[:],
                                     start=True, stop=False)
                    for d in range(ND):
                        nc.tensor.matmul(pv[:], xsb[:, d, i * P:(i + 1) * P],
                                         wv_sb[:, d, :],
                                         start=False, stop=(d == ND - 1))
                    nc.vector.tensor_copy(
                        v16[:, i, :, 0:HD],
                        pv[:].rearrange("p (h w) -> p h w", h=HPC))

            # ---------------- Phase B/C: attention ----------------
            with (
                tc.tile_pool(name="m_sb", bufs=2) as m_pool,
                tc.tile_pool(name="pipe", bufs=4) as pipe,
                tc.tile_pool(name="rpool", bufs=4) as rpool,
                tc.tile_pool(name="ps_sc", bufs=2, space="PSUM") as ps_sc,
                tc.tile_pool(name="ps_out", bufs=1, space="PSUM") as ps_out,
                tc.tile_pool(name="ps_b", bufs=1, space="PSUM") as ps_b,
            ):
                m16_ap = m16.rearrange("(i p) (j q) -> p i j q", p=P, q=512)
                for j in range(NJ):
                    # M^T tiles, duplicated per head-in-pair so elementwise
                    # ops get dense [128, 1024] access patterns.
                    mj = m_pool.tile([P, NI, 2, 512], f16)
                    for g in range(4):
                        for dup in range(2):
                            nc.sync.dma_start(mj[:, 4 * g:4 * g + 4, dup, :],
                                              m16_ap[:, 4 * g:4 * g + 4, j, :])
                    for hp in range(2):  # head pairs: heads (2hp, 2hp+1)
                        p2a = ps_out.tile([HD + 1, 512], f32, tag="p2a")
                        p2b = ps_out.tile([HD + 1, 512], f32, tag="p2b")
                        p2 = [p2a, p2b]
                        for i in range(NI):
                            p1 = ps_sc.tile([P, 2, 512], f32)
                            for hh in range(2):
                                h = 2 * hp + hh
                                pp, half = divmod(h, 2)
                                rows = slice(64 * half, 64 * half + 64)
                                nc.tensor.matmul(
                                    p1[:, hh, :],
                                    kt[rows, pp, i * P:(i + 1) * P],
                                    qt[rows, pp, j * 512:(j + 1) * 512],
                                    start=True, stop=True)
                            t16 = pipe.tile([P, 2, 512], f16, tag="t")
                            nc.vector.tensor_tensor(
                                t16[:], p1[:], mj[:, i, :, :], op=Alu.mult)
                            e16 = pipe.tile([P, 2, 512], f16, tag="e")
                            nc.scalar.activation(e16[:], t16[:], Act.Exp)
                            s16 = pipe.tile([P, 2, 512], f16, tag="s")
                            nc.vector.tensor_tensor(
                                s16[:], e16[:], mj[:, i, :, :], op=Alu.mult)
                            for hh in range(2):
                                h = 2 * hp + hh
                                nc.tensor.matmul(
                                    p2[hh][:], v16[:, i, h, :], s16[:, hh, :],
                                    start=(i == 0), stop=(i == NI - 1))
                        # Phase C: renormalize rows and store y^T (fp16)
                        for hh in range(2):
                            h = 2 * hp + hh
                            pp, half = divmod(h, 2)
                            rows = slice(64 * half, 64 * half + 64)
                            rinv = rpool.tile([1, 512], f32)
                            nc.vector.reciprocal(rinv[:], p2[hh][HD:HD + 1, :])
                            pb = ps_b.tile([HD, 512], f32)
                            nc.tensor.matmul(pb[:], ones[0:1, 0:HD], rinv[:],
                                             start=True, stop=True)
                            rb = rpool.tile([HD, 512], f32, tag="rb")
                            nc.scalar.activation(rb[:], pb[:], Act.Copy)
                            nc.vector.tensor_tensor(
                                y_sb[rows, pp, j * 512:(j + 1) * 512],
                                p2[hh][0:HD, :], rb[:], op=Alu.mult)

            # ---------------- Phase D: output projection ----------------
            with (
                tc.tile_pool(name="wo_sb", bufs=1) as wo_pool,
                tc.tile_pool(name="o_sb", bufs=3) as o_pool,
                tc.tile_pool(name="psD", bufs=2, space="PSUM") as psD,
            ):
                wo_sb = wo_pool.tile([P, 2, D], f16)
                nc.sync.dma_start(wo_sb[:], wo.rearrange("c p n -> p c n"))
                for jq in range(S // P):
                    o_sb = o_pool.tile([P, D], f32)
                    for nb in range(2):
                        p4 = psD.tile([P, 512], f32)
                        for cc in range(2):
                            nc.tensor.matmul(
                                p4[:], y_sb[:, cc, jq * P:(jq + 1) * P],
                                wo_sb[:, cc, nb * 512:(nb + 1) * 512],
                                start=(cc == 0), stop=(cc == 1))
                        nc.scalar.activation(o_sb[:, nb * 512:(nb + 1) * 512],
                                             p4[:], Act.Copy)
                    nc.sync.dma_start(out[jq * P:(jq + 1) * P, :], o_sb[:])
    nc.compile()
    return nc


def make_core_inputs(x, M, Wq, bq_, Wk, bk_, Wv, bv_, Wo, bo_):
    """Build the 8 per-core input maps (host-side sharding)."""
    sm = float(HD) ** -0.5
    maps = []
    mt16 = {}
    xT16 = {}
    for b in range(B):
        mt16[b] = np.ascontiguousarray(M[b].T).astype(np.float16)
        xT16[b] = np.ascontiguousarray(x[b].T).astype(np.float16).reshape(ND, P, S)
    for c in range(NCORES):
        b, hg = divmod(c, GROUPS)
        rows = slice(WROWS * hg, WROWS * (hg + 1))
        im = {
            "xT": xT16[b],
            "m16": mt16[b],
            "wq": np.ascontiguousarray((Wq[rows] * sm).T).astype(np.float16).reshape(ND, P, WROWS),
            "wk": np.ascontiguousarray(Wk[rows].T).astype(np.float16).reshape(ND, P, WROWS),
            "wv": np.ascontiguousarray(Wv[rows].T).astype(np.float16).reshape(ND, P, WROWS),
            "bq": (bq_[rows] * sm).reshape(2, P).astype(np.float32),
            "bk": bk_[rows].reshape(2, P).astype(np.float32),
            "bv": bv_[rows].reshape(1, WROWS).astype(np.float32),
            "wo": np.ascontiguousarray(Wo[:, rows].T).astype(np.float16).reshape(2, P, D),
        }
        maps.append({k: np.ascontiguousarray(v) for k, v in im.items()})
    return maps


def kernel(x, M, Wq, bq, Wk, bk, Wv, bv, Wo, bo, _trace=False):
    from concourse.bass_utils import run_bass_kernel_spmd

    if "nc" not in _CACHE:
        _CACHE["nc"] = _build_nc()
    nc = _CACHE["nc"]

    in_maps = make_core_inputs(np.asarray(x, np.float32), np.asarray(M, np.float32),
                               np.asarray(Wq, np.float32), np.asarray(bq, np.float32),
                               np.asarray(Wk, np.float32), np.asarray(bk, np.float32),
                               np.asarray(Wv, np.float32), np.asarray(bv, np.float32),
                               np.asarray(Wo, np.float32), np.asarray(bo, np.float32))
    res = run_bass_kernel_spmd(nc, in_maps, list(range(NCORES)), trace=_trace)
    out = np.zeros((B, S, D), np.float32)
    for c in range(NCORES):
        b = c // GROUPS
        out[b] += res.results[c]["out"]
    out += np.asarray(bo, np.float32)
    if _trace:
        _CACHE["last_result"] = res
    return out
